# revision 22
# baseline (speedup 1.0000x reference)
"""Trainium2 Bass kernel for nn_ComplexDotProduct.

  out[b, o, n] = sum_c complex(x)[b, c, n] * complex(w)[o, c, n] + bias[o]
  B=64, C=128, N=1024, O=512.

Strategy
--------
Shard N across the 8 cores (128 positions each) — no tensor is replicated,
so per-core HBM traffic is the global minimum.

The 2e-2 rel-err budget admits bf16 operands and outputs (measured 2.9e-3),
halving HBM bytes vs fp32: per core ~37.8 MB in + ~16.8 MB out (vs ~113 MB
for the fp32 version).  The kernel is HBM-bound — 8 cores together sustain
~2.5 TB/s of the chip's 2.86 TB/s, so halving bytes is ~2x end to end
(380us -> ~171us).  Measured engine occupancy: DMA-only ~171us,
PE+DVE ~80us, so compute hides entirely under the HBM stream.

Per position n the computation is a complex matmul
  [C=128, B=64]^T @ [C=128, O=512]  (4 real matmuls per position)
with x stationary (M=64) and w moving (512 columns, one PSUM bank).
out_re = x_re.T@w_re + (-x_im).T@w_im   (x_im negated on-chip by DVE),
out_im = x_im.T@w_re + x_re.T@w_im.

Positions are processed in (even, odd) pairs sharing PSUM banks: the even
position's matmuls write PSUM partitions 0-63 (PE column group 0-1), the
odd position's write partitions 64-127 (column group 2-3) — legal for bf16
(only fp32r is restricted to base partition 0).  Matmul instructions
alternate between the two halves so adjacent instructions target disjoint
PE column groups and stream concurrently (measured PE+DVE 150us -> 80us),
with the stationary x operand shared across the re/im streams of a phase.
One pair of [128, 512] DVE tensor_tensor ops then evacuates re/im with the
bias fused, and the store runs 128 partitions wide across all 16 SDMA
engines.  Loads own the SP HWDGE ring and stores own the ACT ring, so a
compute-gated store never head-of-line blocks a ready load.

Host-side prep packs one bf16 input stream a[(C, N, 1152)] =
[w_re(512) | w_im(512) | x_re(64) | x_im(64)] per (c, n) so every DMA is
long-contiguous per partition; the kernel writes out as
(128, NSH/2, 2, O) bf16 per core and the host assembles complex64
(B, O, N).
"""

import numpy as np
import ml_dtypes

B, C, N, O = 64, 128, 1024, 512
NCORES = 8
NSH = N // NCORES        # 128 positions per core
JT = 8                   # positions per j-tile
NT = NSH // JT           # 16 j-tiles per core
AW = 2 * O + 2 * B       # 1152: packed row [w_re | w_im | x_re | x_im]
XR = 2 * O               # x_re offset
XI = 2 * O + B           # x_im offset

BF16 = ml_dtypes.bfloat16


def build_nc(loop_r=None, timing_pool=None, parts="all", jt=None, bufs=(4, 3),
             store="sep", psb=3, se=1, bodies=1):
    """Build the per-core Tile program.

    loop_r: wrap the body in a hardware For_i loop (timing only).
    timing_pool: if set (e.g. 2), DRAM in/out tensors cover only that many
    j-tiles and the body cycles through them — keeps the uploaded bytes tiny
    for loop-delta timing while preserving per-iteration DMA/compute work.
    parts: "all" | "dma" (skip compute) | "noout" (skip output store)
           | "nodma" (compute only, from a resident SBUF tile).
    store: "sep" (loads on SP ring, stores on ACT ring) | "alt"
           (alternate rings per j-tile) | "split" (half on each ring)
           | "gpsimd" (SWDGE ring).
    """
    import concourse.mybir as mybir
    from concourse import bacc
    from concourse.tile import TileContext

    bf16 = mybir.dt.bfloat16
    f32 = mybir.dt.float32
    add = mybir.AluOpType.add

    nc = bacc.Bacc(None, target_bir_lowering=False, debug=False)

    jt = JT if jt is None else jt
    nt = NSH // jt
    pool_n = NSH if timing_pool is None else timing_pool * jt
    a_d = nc.dram_tensor("a", (C, pool_n, AW), bf16, kind="ExternalInput")
    b_d = nc.dram_tensor("bt", (2 * B, 2, O), f32, kind="ExternalInput")
    out_d = nc.dram_tensor("out", (2 * B, pool_n // 2, 2, O), bf16,
                           kind="ExternalOutput")

    with TileContext(nc) as tc:
        with (
            tc.tile_pool(name="xw", bufs=bufs[0]) as xw,
            tc.tile_pool(name="ob", bufs=bufs[1]) as ob,
            tc.tile_pool(name="cst", bufs=1) as cst,
            tc.tile_pool(name="ps", bufs=psb, space="PSUM") as ps,
        ):
            b_t = cst.tile([2 * B, 2, O], f32)
            nc.sync.dma_start(out=b_t[:], in_=b_d[:])

            def one_pair(a_t, xn_t, o_t, p, po=None):
                po = p if po is None else po
                # positions (2p, 2p+1): even -> PSUM parts 0-63 (PE col
                # group 0-1), odd -> parts 64-127 (col group 2-3).  Matmuls
                # alternate halves so adjacent instructions target disjoint
                # column groups and can run concurrently; within a phase the
                # stationary operand is shared by the re and im streams.
                ps_re = ps.tile([2 * B, O], mybir.dt.float32, name="ps_re")
                ps_im = ps.tile([2 * B, O], mybir.dt.float32, name="ps_im")
                HS = ((0, slice(0, B)), (1, slice(B, 2 * B)))
                # phase 1: stationary x_re -> re += x_re@w_re (start),
                #                            im += x_re@w_im (start)
                for mm in range(2):
                    for h, sl in HS:
                        j = 2 * p + h
                        bank, woff = ((ps_re, 0), (ps_im, O))[mm]
                        nc.tensor.matmul(bank[sl], a_t[:, j, XR:XR + B],
                                         a_t[:, j, woff:woff + O],
                                         start=True, stop=False)
                # phases 2+3 (independent): im += x_im@w_re (stop),
                #                           re += -x_im@w_im (stop)
                for mm in range(2):
                    for h, sl in HS:
                        j = 2 * p + h
                        if (mm + h) % 2 == 0:
                            nc.tensor.matmul(ps_im[sl], a_t[:, j, XI:XI + B],
                                             a_t[:, j, 0:O],
                                             start=False, stop=True)
                        else:
                            nc.tensor.matmul(ps_re[sl], xn_t[:, j, :],
                                             a_t[:, j, O:2 * O],
                                             start=False, stop=True)
                nc.vector.tensor_tensor(o_t[:, po, 0, :], ps_re[:],
                                        b_t[:, 0, :], add)
                nc.vector.tensor_tensor(o_t[:, po, 1, :], ps_im[:],
                                        b_t[:, 1, :], add)

            if parts == "nodma":
                ra_t = cst.tile([C, jt, AW], bf16)
                rxn_t = cst.tile([C, jt, B], bf16)
                nc.vector.memset(ra_t[:], 0.001)
                nc.vector.memset(rxn_t[:], 0.001)

            def body(_i=None):
                o_t = None
                for jt_i in range(nt):
                    if jt_i % se == 0:
                        o_t = ob.tile([2 * B, se * jt // 2, 2, O], bf16,
                                      name="o_t")
                    ob_off = (jt_i % se) * (jt // 2)
                    eff = jt_i if timing_pool is None else jt_i % timing_pool
                    if parts == "nodma":
                        for p in range(jt // 2):
                            one_pair(ra_t, rxn_t, o_t, p, ob_off + p)
                        continue
                    a_t = xw.tile([C, jt, AW], bf16, name="a_t")
                    xn_t = xw.tile([C, jt, B], bf16, name="xn_t")
                    sl = slice(eff * jt, (eff + 1) * jt)
                    h = jt // 2
                    if store == "sep":
                        # loads own the SP ring; stores own the ACT ring —
                        # a compute-gated store can then never head-of-line
                        # block a ready load on the same HWDGE sequencer.
                        nc.sync.dma_start(out=a_t[:], in_=a_d[:, sl])
                    else:
                        nc.sync.dma_start(out=a_t[:, :h], in_=a_d[:, sl][:, :h])
                        nc.scalar.dma_start(out=a_t[:, h:], in_=a_d[:, sl][:, h:])
                    if parts != "dma":
                        # split per DMA half so pair 0 isn't gated on both
                        nc.vector.tensor_scalar_mul(
                            xn_t[:, :h], a_t[:, :h, XI:XI + B], -1.0)
                        nc.vector.tensor_scalar_mul(
                            xn_t[:, h:], a_t[:, h:, XI:XI + B], -1.0)
                        for p in range(jt // 2):
                            one_pair(a_t, xn_t, o_t, p, ob_off + p)
                    else:
                        nc.vector.memset(o_t[0:1, 0, 0, 0:1], 0.0)
                    if parts != "noout" and jt_i % se == se - 1:
                        k = se * jt // 2
                        o0 = (eff - (se - 1)) * jt // 2
                        osl = slice(o0, o0 + k)
                        if store == "split":
                            nc.sync.dma_start(out=out_d[:, osl][:, :k // 2],
                                              in_=o_t[:, :k // 2])
                            nc.scalar.dma_start(out=out_d[:, osl][:, k // 2:],
                                                in_=o_t[:, k // 2:])
                        elif store == "sep":
                            nc.scalar.dma_start(out=out_d[:, osl], in_=o_t[:])
                        elif store == "gpsimd":
                            nc.gpsimd.dma_start(out=out_d[:, osl], in_=o_t[:])
                        else:
                            eng = nc.scalar if (jt_i // se) % 2 else nc.sync
                            eng.dma_start(out=out_d[:, osl], in_=o_t[:])

            if loop_r is None:
                for _ in range(bodies):
                    body()
            else:
                with tc.For_i(0, loop_r, 1):
                    for _ in range(bodies):
                        body()

    nc.compile()
    return nc


def _prep_inputs(x_re, x_im, w_re, w_im, b_re, b_im):
    """Host-side packing into the kernel's DMA-friendly bf16 layout.
    Threaded over blocks to speed up the big w transpose + cast."""
    from concurrent.futures import ThreadPoolExecutor

    x_re = np.asarray(x_re, dtype=np.float32)
    x_im = np.asarray(x_im, dtype=np.float32)
    w_re = np.asarray(w_re, dtype=np.float32)
    w_im = np.asarray(w_im, dtype=np.float32)
    b_re = np.asarray(b_re, dtype=np.float32)
    b_im = np.asarray(b_im, dtype=np.float32)

    # a4: (core, C, NSH, AW) bf16 <- [w_re, w_im, x_re, x_im]
    a4 = np.empty((NCORES, C, NSH, AW), BF16)

    def do_w(args):
        k, c0, core = args
        src = w_re[0] if k == 0 else w_im[0]
        nsl = slice(core * NSH, (core + 1) * NSH)
        # dst (cblk, NSH, O) <- src (O, cblk, NSH)
        a4[core, c0:c0 + 16, :, k * O:(k + 1) * O] = \
            src[:, c0:c0 + 16, nsl].transpose(1, 2, 0).astype(BF16)

    def do_x(args):
        k, core = args
        src = x_re if k == 0 else x_im
        nsl = slice(core * NSH, (core + 1) * NSH)
        a4[core, :, :, XR + k * B:XR + (k + 1) * B] = \
            src[:, :, nsl].transpose(1, 2, 0).astype(BF16)

    with ThreadPoolExecutor(max_workers=16) as ex:
        futs = [ex.submit(do_w, (k, c0, core)) for k in range(2)
                for c0 in range(0, C, 16) for core in range(NCORES)]
        futs += [ex.submit(do_x, (k, core)) for k in range(2)
                 for core in range(NCORES)]
        for f in futs:
            f.result()

    bt = np.empty((2 * B, 2, O), np.float32)
    bt[:, 0, :] = b_re[0, :, 0][None, :]
    bt[:, 1, :] = b_im[0, :, 0][None, :]

    return [{"a": a4[c], "bt": bt} for c in range(NCORES)]


def _assemble(results):
    """Per-core 'out' buffers -> (B, O, N) complex64."""
    from concurrent.futures import ThreadPoolExecutor

    out = np.empty((B, O, N), np.complex64)

    def do_core(c):
        buf = results[c]["out"].astype(np.float32)  # (128, NSH/2, 2, O)
        blk = out[:, :, c * NSH:(c + 1) * NSH]
        # parts 0-63: even positions, parts 64-127: odd positions
        blk[:, :, 0::2] = (buf[:B, :, 0, :] + 1j * buf[:B, :, 1, :]) \
            .transpose(0, 2, 1)
        blk[:, :, 1::2] = (buf[B:, :, 0, :] + 1j * buf[B:, :, 1, :]) \
            .transpose(0, 2, 1)

    with ThreadPoolExecutor(max_workers=8) as ex:
        list(ex.map(do_core, range(NCORES)))
    return out


def kernel(x_re, x_im, w_re, w_im, b_re, b_im):
    from concourse import bass_utils

    nc = build_nc()
    in_maps = _prep_inputs(x_re, x_im, w_re, w_im, b_re, b_im)
    res = bass_utils.run_bass_kernel_spmd(nc, in_maps, core_ids=list(range(NCORES)))
    return _assemble(res.results)
